# revision 29
# baseline (speedup 1.0000x reference)
"""Trainium2 Bass kernel for the KSubspaceBaseModel objective.

Reference computes, for B=2048 samples x (B, D=1024) and subspace bases
Us (R=4, K=16, D, d=32):
    z = x @ U; x_ = z @ U^T; loss = 0.5*||x - x_||^2  (per b, r, k)
    obj_r = mean_b min_k loss

Algebraic collapse: with G = U^T U and L = chol(I - 0.5 G) folded host-side
(Ut = U @ L), loss = 0.5||x||^2 - ||Ut^T x||^2, so the device computes
z~ = Ut^T x, squares, and sums each subspace's 32 latent columns; the
per-k sums ship out and the host takes max_k and the fp64 base term.

Device speed notes:
  * fp8 e4m3 operands (Ut scaled by 4096 to stay in the normal range) with
    DoubleRow matmuls: each instruction consumes TWO 128-deep contraction
    chunks ([128, 2, M] stationary x [128, 2, N] moving), 2x PE throughput
    and half the DMA bytes.
  * COLUMN-major u chunking in CONSUMPTION order: each 256 KB u transfer
    carries all 4 contraction pairs for a 256-column block, and the sync
    ring's FIFO [u0c0, u0c1, u1c0, u1c1] means every chunk that lands
    unlocks 16 matmuls immediately (DMA engines round-robin across rings,
    so in-flight transfers finish roughly in issue order).
  * one PSUM bank per (nh, bc) group: matmuls never wait on an epilogue
    reader, keeping the stream gap-free so the PE p-state ramp
    (0.65 -> 1.2 -> 2.4 GHz after a few us continuous) works for us; fp8
    warm-up matmuls bridge the DMA head.
  * epilogue spread over three engines: ScalarE squares (PSUM -> bf16),
    GpSimd sums replicate-0 groups, DVE sums replicate-1 groups per
    half-block so the last chunk's tail is short; k-max and batch-mean
    happen on host from the shipped [128, 8*16] per-k sums.
"""

import numpy as np
import ml_dtypes

import concourse.bass as bass
import concourse.bacc as bacc
import concourse.mybir as mybir
import concourse.tile as tile
from concourse.bass_utils import run_bass_kernel_spmd

B, D, R, K, d = 2048, 1024, 4, 16, 32
NCORES = 8
NB = B // 4          # 512 samples per core
BC = NB // 128       # 4 batch chunks per core
NJ = 4               # contraction pairs (8 kc chunks, 2 per DoubleRow matmul)
SCALE = 4096.0       # Ut pre-scale so fp8 e4m3 values are normal-range
WARM = 15            # PE warm-up matmuls (p-state ramp during DMA head)

FP8 = mybir.dt.float8e4
BF16 = mybir.dt.bfloat16
FP32 = mybir.dt.float32

_COMPILED = {}
LAST_RESULTS = None


def _build():
    nc = bacc.Bacc("TRN2", target_bir_lowering=False, debug=False)
    # host pre-arranges every tensor into its exact SBUF image so each
    # partition's DMA read is one contiguous run
    xt = nc.dram_tensor("xt", [128, BC * NJ * 2 * 128], FP8,
                        kind="ExternalInput")        # [p, bc, j, i, b]
    u0 = nc.dram_tensor("u0", [128, 2 * NJ * 2 * 256], FP8,
                        kind="ExternalInput")        # [p, cb, j, i, c]
    u1 = nc.dram_tensor("u1", [128, 2 * NJ * 2 * 256], FP8,
                        kind="ExternalInput")
    outp = nc.dram_tensor("outp", [128, 128], FP32, kind="ExternalOutput")

    xt_v = xt.ap().rearrange("p (b j i n) -> p b j i n", b=BC, j=NJ, i=2)
    u_v = [u.ap().rearrange("p (cb j i c) -> p cb j i c", cb=2, j=NJ, i=2)
           for u in (u0, u1)]
    DR = mybir.MatmulPerfMode.DoubleRow

    # phase order = chunk consumption order on the sync ring
    PHASES = [(0, 0), (0, 1), (1, 0), (1, 1)]

    with tile.TileContext(nc) as tc:
        with (
            tc.tile_pool(name="xsb", bufs=1) as xpool,
            tc.tile_pool(name="usb", bufs=1) as upool,
            tc.tile_pool(name="esb", bufs=1) as epool,
            tc.tile_pool(name="single", bufs=1) as spool,
            tc.tile_pool(name="zp", bufs=1, space="PSUM") as zpool,
        ):
            # x lands as ONE 512 KB transfer: all batch chunks unblock at
            # once, so the scheduler sees the four bc-groups of every phase
            # as symmetric and orders the epilogue squares early (their
            # semaphore positions then stay small). u chunks go in
            # consumption order; the last is split so it gates only 8
            # matmuls.
            xb = [xpool.tile([128, 2, NJ, 2, 128], FP8, tag=f"x{bp}",
                             name=f"x{bp}") for bp in range(2)]
            u00 = upool.tile([128, NJ, 2, 256], FP8, tag="u00", name="u00")
            u01 = upool.tile([128, NJ, 2, 256], FP8, tag="u01", name="u01")
            u10 = upool.tile([128, NJ, 2, 256], FP8, tag="u10", name="u10")
            u11 = [upool.tile([128, 2, 2, 256], FP8, tag=f"u11{h}",
                              name=f"u11{h}") for h in range(2)]

            def uview(nh, cb, j):
                if (nh, cb) == (0, 0):
                    return u00[:, j]
                if (nh, cb) == (0, 1):
                    return u01[:, j]
                if (nh, cb) == (1, 0):
                    return u10[:, j]
                return u11[j // 2][:, j % 2]

            def xview(bc, j):
                return xb[bc // 2][:, bc % 2, j]

            warm = spool.tile([128, 2, 384], FP8, tag="warm")
            nc.gpsimd.memset(warm[:], 0.0)

            # scalar ring: x as two 256 KB pair-chunks (first u-phase's
            # early groups unblock with x01); sync ring: u chunks in
            # consumption order, the last split so it gates only 8 matmuls
            nc.scalar.dma_start(xb[0][:], xt_v[:, 0:2])
            nc.scalar.dma_start(xb[1][:], xt_v[:, 2:4])
            nc.sync.dma_start(u00[:], u_v[0][:, 0])
            nc.sync.dma_start(u01[:], u_v[0][:, 1])
            nc.sync.dma_start(u10[:], u_v[1][:, 0])
            nc.sync.dma_start(u11[0][:], u_v[1][:, 1, 0:2])
            nc.sync.dma_start(u11[1][:], u_v[1][:, 1, 2:4])

            # per-k sums land here slice-wise: group g = nh*4+bc covers
            # cols [g*16, (g+1)*16)
            ostage = spool.tile([128, 128], FP32, tag="os", name="ostage")

            # PSUM bank keyed (cb, bc), region keyed nh: phases alternate
            # banks (P1/P3 share a bank, P2/P4 the other), so the only
            # write-after-read pair on a bank is two phases apart and its
            # quantized semaphore wait is satisfied long before it matters
            zps = {(cb, bc): zpool.tile([128, 512], FP32,
                                        tag=f"zp{cb}{bc}",
                                        name=f"zp{cb}{bc}")
                   for cb in range(2) for bc in range(BC)}

            # warm-up: keep the PE busy while the first chunks stream in so
            # the p-state ramp clock starts early (writes bank (0,0); the
            # real group 0 resets it with start=True)
            for _ in range(WARM):
                nc.tensor.matmul(zps[(0, 0)][:, 0:256], warm[:, :, 0:128],
                                 warm[:, :, 128:384], start=True, stop=True,
                                 perf_mode=DR, skip_group_check=True)

            et = {(nh, bc): epool.tile([128, 512], BF16, tag=f"e{nh}{bc}",
                                       name=f"e{nh}{bc}")
                  for nh in range(2) for bc in range(BC)}

            for nh, cb in PHASES:
                sl = slice(cb * 256, (cb + 1) * 256)
                psl = slice(nh * 256, (nh + 1) * 256)
                for bc in range(BC):
                    g = nh * 4 + bc
                    for j in range(NJ):
                        nc.tensor.matmul(
                            zps[(cb, bc)][:, psl], xview(bc, j),
                            uview(nh, cb, j),
                            start=(j == 0), stop=(j == NJ - 1),
                            perf_mode=DR, skip_group_check=True)
                    # epilogue at priority 0: the scheduler places each op
                    # immediately after its producer, so the cross-engine
                    # semaphore targets stay minimal even though the cost
                    # model mispredicts matmul/DMA durations
                    with tc.high_priority(offset=10000):
                        nc.scalar.square(et[(nh, bc)][:, sl],
                                         zps[(cb, bc)][:, psl])
                        if nh == 0 and cb == 1:
                            # replicate-0 groups complete here: one 512-col
                            # subspace-sum per group
                            nc.vector.reduce_sum(
                                ostage[:, g * K:(g + 1) * K],
                                et[(nh, bc)].rearrange(
                                    "p (k c) -> p k c", c=d),
                                axis=mybir.AxisListType.X)
                            if bc == BC - 1:
                                nc.sync.dma_start(outp.ap()[:, 0:64],
                                                  ostage[:, 0:64])
                        elif nh == 1:
                            # replicate-1: per-half sums on DVE right after
                            # each square so the final chunk's tail is short
                            nc.vector.reduce_sum(
                                ostage[:, g * K + cb * 8:g * K + cb * 8 + 8],
                                et[(nh, bc)][:, sl].rearrange(
                                    "p (k c) -> p k c", c=d),
                                axis=mybir.AxisListType.X)
            with tc.high_priority(offset=10000):
                nc.sync.dma_start(outp.ap()[:, 64:128], ostage[:, 64:128])

    nc.compile()
    return nc


def _prep(x, Us):
    # fold chol(I - 0.5 U^T U) into U, then scale+quantize to fp8 e4m3
    Us64 = Us.astype(np.float64)
    G = np.einsum('skDa,skDb->skab', Us64, Us64)
    L = np.linalg.cholesky(np.eye(d)[None, None] - 0.5 * G)
    Ut = np.einsum('skDa,skab->skDb', Us64, L)                # (R,K,D,d)
    u8 = (Ut * SCALE).astype(np.float32).astype(ml_dtypes.float8_e4m3)
    x8 = np.ascontiguousarray(x.T).astype(ml_dtypes.float8_e4m3)  # (D, B)

    def u_img(r):  # one replicate -> [128, 2*NJ*2*256] (p, cb, j, i, c)
        ur = np.ascontiguousarray(u8[r].transpose(1, 0, 2)).reshape(D, K * d)
        return np.ascontiguousarray(
            ur.reshape(NJ, 2, 128, 2, 256).transpose(2, 3, 0, 1, 4)
        ).reshape(128, 2 * NJ * 2 * 256)

    def x_img(b4):  # one batch quarter -> [128, BC*NJ*2*128] (p, bc, j, i, n)
        xc = x8[:, NB * b4: NB * (b4 + 1)]                    # (D, 512)
        return np.ascontiguousarray(
            xc.reshape(NJ, 2, 128, BC, 128).transpose(2, 3, 0, 1, 4)
        ).reshape(128, BC * NJ * 2 * 128)

    u_imgs = [u_img(r) for r in range(R)]
    x_imgs = [x_img(b4) for b4 in range(BC)]
    in_maps = []
    for c in range(NCORES):
        s2, b4 = c // 4, c % 4
        in_maps.append({
            "xt": x_imgs[b4],
            "u0": u_imgs[2 * s2],
            "u1": u_imgs[2 * s2 + 1],
        })
    return in_maps


def kernel(x, Us, _trace=False):
    global LAST_RESULTS
    x = np.asarray(x)
    Us = np.asarray(Us)
    if "nc" not in _COMPILED:
        _COMPILED["nc"] = _build()
    nc = _COMPILED["nc"]
    in_maps = _prep(x, Us)
    res = run_bass_kernel_spmd(nc, in_maps, core_ids=list(range(NCORES)),
                               trace=_trace)
    LAST_RESULTS = res
    # base term: exact fp64 sum of squares on host (tiny vs the device work)
    base = 0.5 * float(np.sum(x.astype(np.float64) ** 2)) / B
    obj = np.empty(R, np.float32)
    for r in range(R):
        s2, nh = r // 2, r % 2
        # group col block = (nh*4 + bc)*16; z~ was scaled by SCALE
        terms = []
        for b4 in range(4):
            o = res.results[4 * s2 + b4]["outp"]              # [128, 128]
            for bc in range(BC):
                g = nh * 4 + bc
                terms.append(o[:, g * K:(g + 1) * K].astype(np.float64)
                             .max(axis=1))
        term = np.mean(np.stack(terms)) / (SCALE * SCALE)
        obj[r] = np.float32(base - term)
    return obj


# revision 32
# speedup vs baseline: 1.1389x; 1.1389x over previous
"""Trainium2 Bass kernel for the KSubspaceBaseModel objective.

Reference computes, for B=2048 samples x (B, D=1024) and subspace bases
Us (R=4, K=16, D, d=32):
    z = x @ U; x_ = z @ U^T; loss = 0.5*||x - x_||^2  (per b, r, k)
    obj_r = mean_b min_k loss

Algebraic collapse: with G = U^T U and L = chol(I - 0.5 G) folded host-side
(Ut = U @ L), loss = 0.5||x||^2 - ||Ut^T x||^2, so the device computes
z~ = Ut^T x, squares, and sums each subspace's 32 latent columns; the
per-k sums ship out and the host takes max_k and the fp64 base term.

Device speed notes:
  * fp8 e4m3 operands (Ut scaled by 4096 to stay in the normal range) with
    DoubleRow matmuls: each instruction consumes TWO 128-deep contraction
    chunks ([128, 2, M] stationary x [128, 2, N] moving), 2x PE throughput
    and half the DMA bytes.
  * COLUMN-major u chunking in CONSUMPTION order: each 256 KB u transfer
    carries all 4 contraction pairs for a 256-column block, and the sync
    ring's FIFO [u0c0, u0c1, u1c0, u1c1] means every chunk that lands
    unlocks 16 matmuls immediately (DMA engines round-robin across rings,
    so in-flight transfers finish roughly in issue order).
  * one PSUM bank per (nh, bc) group: matmuls never wait on an epilogue
    reader, keeping the stream gap-free so the PE p-state ramp
    (0.65 -> 1.2 -> 2.4 GHz after a few us continuous) works for us; fp8
    warm-up matmuls bridge the DMA head.
  * epilogue spread over three engines: ScalarE squares (PSUM -> bf16),
    GpSimd sums replicate-0 groups, DVE sums replicate-1 groups per
    half-block so the last chunk's tail is short; k-max and batch-mean
    happen on host from the shipped [128, 8*16] per-k sums.
"""

import numpy as np
import ml_dtypes

import concourse.bass as bass
import concourse.bacc as bacc
import concourse.mybir as mybir
import concourse.tile as tile
from concourse.bass_utils import run_bass_kernel_spmd

B, D, R, K, d = 2048, 1024, 4, 16, 32
NCORES = 8
NB = B // 4          # 512 samples per core
BC = NB // 128       # 4 batch chunks per core
NJ = 4               # contraction pairs (8 kc chunks, 2 per DoubleRow matmul)
SCALE = 4096.0       # Ut pre-scale so fp8 e4m3 values are normal-range
WARM = 15            # PE warm-up matmuls (p-state ramp during DMA head)

FP8 = mybir.dt.float8e4
BF16 = mybir.dt.bfloat16
FP32 = mybir.dt.float32

_COMPILED = {}
LAST_RESULTS = None


def _build():
    nc = bacc.Bacc("TRN2", target_bir_lowering=False, debug=False)
    # host pre-arranges every tensor into its exact SBUF image so each
    # partition's DMA read is one contiguous run
    xt = nc.dram_tensor("xt", [128, BC * NJ * 2 * 128], FP8,
                        kind="ExternalInput")        # [p, bc, j, i, b]
    u0 = nc.dram_tensor("u0", [128, 2 * NJ * 2 * 256], FP8,
                        kind="ExternalInput")        # [p, cb, j, i, c]
    u1 = nc.dram_tensor("u1", [128, 2 * NJ * 2 * 256], FP8,
                        kind="ExternalInput")
    outp = nc.dram_tensor("outp", [128, 128], FP32, kind="ExternalOutput")

    xt_v = xt.ap().rearrange("p (b j i n) -> p b j i n", b=BC, j=NJ, i=2)
    u_v = [u.ap().rearrange("p (cb j i c) -> p cb j i c", cb=2, j=NJ, i=2)
           for u in (u0, u1)]
    DR = mybir.MatmulPerfMode.DoubleRow

    # phase order = chunk consumption order on the sync ring
    PHASES = [(0, 0), (0, 1), (1, 0), (1, 1)]

    with tile.TileContext(nc) as tc:
        with (
            tc.tile_pool(name="xsb", bufs=1) as xpool,
            tc.tile_pool(name="usb", bufs=1) as upool,
            tc.tile_pool(name="esb", bufs=1) as epool,
            tc.tile_pool(name="single", bufs=1) as spool,
            tc.tile_pool(name="zp", bufs=1, space="PSUM") as zpool,
        ):
            # x lands as ONE 512 KB transfer: all batch chunks unblock at
            # once, so the scheduler sees the four bc-groups of every phase
            # as symmetric and orders the epilogue squares early (their
            # semaphore positions then stay small). u chunks go in
            # consumption order; the last is split so it gates only 8
            # matmuls.
            xb = xpool.tile([128, BC, NJ, 2, 128], FP8, tag="x", name="xb")
            u00 = upool.tile([128, NJ, 2, 256], FP8, tag="u00", name="u00")
            u01 = upool.tile([128, NJ, 2, 256], FP8, tag="u01", name="u01")
            u10 = upool.tile([128, NJ, 2, 256], FP8, tag="u10", name="u10")
            u11 = [upool.tile([128, 2, 2, 256], FP8, tag=f"u11{h}",
                              name=f"u11{h}") for h in range(2)]

            def uview(nh, cb, j):
                if (nh, cb) == (0, 0):
                    return u00[:, j]
                if (nh, cb) == (0, 1):
                    return u01[:, j]
                if (nh, cb) == (1, 0):
                    return u10[:, j]
                return u11[j // 2][:, j % 2]

            def xview(bc, j):
                return xb[:, bc, j]

            warm = spool.tile([128, 2, 384], FP8, tag="warm")
            nc.gpsimd.memset(warm[:], 0.0)

            # scalar ring: x as one 512 KB transfer (all batch chunks
            # unblock at once, keeping group readiness symmetric); sync
            # ring: u chunks in consumption order, the last split so it
            # gates only 8 matmuls
            nc.scalar.dma_start(xb[:], xt_v[:])
            nc.sync.dma_start(u00[:], u_v[0][:, 0])
            nc.sync.dma_start(u01[:], u_v[0][:, 1])
            nc.sync.dma_start(u10[:], u_v[1][:, 0])
            nc.sync.dma_start(u11[0][:], u_v[1][:, 1, 0:2])
            nc.sync.dma_start(u11[1][:], u_v[1][:, 1, 2:4])

            # per-k sums land here slice-wise: group g = nh*4+bc covers
            # cols [g*16, (g+1)*16)
            ostage = spool.tile([128, 128], FP32, tag="os", name="ostage")

            # PSUM bank keyed (cb, bc), region keyed nh: phases alternate
            # banks (P1/P3 share a bank, P2/P4 the other), so the only
            # write-after-read pair on a bank is two phases apart and its
            # quantized semaphore wait is satisfied long before it matters
            zps = {(cb, bc): zpool.tile([128, 512], FP32,
                                        tag=f"zp{cb}{bc}",
                                        name=f"zp{cb}{bc}")
                   for cb in range(2) for bc in range(BC)}

            # warm-up: keep the PE busy while the first chunks stream in so
            # the p-state ramp clock starts early (writes bank (0,0); the
            # real group 0 resets it with start=True)
            for _ in range(WARM):
                nc.tensor.matmul(zps[(0, 0)][:, 0:256], warm[:, :, 0:128],
                                 warm[:, :, 128:384], start=True, stop=True,
                                 perf_mode=DR, skip_group_check=True)

            et = {(nh, bc): epool.tile([128, 512], BF16, tag=f"e{nh}{bc}",
                                       name=f"e{nh}{bc}")
                  for nh in range(2) for bc in range(BC)}

            for nh, cb in PHASES:
                sl = slice(cb * 256, (cb + 1) * 256)
                psl = slice(nh * 256, (nh + 1) * 256)
                for bc in range(BC):
                    g = nh * 4 + bc
                    for j in range(NJ):
                        nc.tensor.matmul(
                            zps[(cb, bc)][:, psl], xview(bc, j),
                            uview(nh, cb, j),
                            start=(j == 0), stop=(j == NJ - 1),
                            perf_mode=DR, skip_group_check=True)
                    # epilogue at priority 0: the scheduler places each op
                    # immediately after its producer, so the cross-engine
                    # semaphore targets stay minimal even though the cost
                    # model mispredicts matmul/DMA durations
                    with tc.high_priority(offset=10000):
                        nc.scalar.square(et[(nh, bc)][:, sl],
                                         zps[(cb, bc)][:, psl])
                        if nh == 0 and cb == 1:
                            # replicate-0 groups complete here: one 512-col
                            # subspace-sum per group
                            nc.vector.reduce_sum(
                                ostage[:, g * K:(g + 1) * K],
                                et[(nh, bc)].rearrange(
                                    "p (k c) -> p k c", c=d),
                                axis=mybir.AxisListType.X)
                            if bc == BC - 1:
                                nc.sync.dma_start(outp.ap()[:, 0:64],
                                                  ostage[:, 0:64])
                        elif nh == 1:
                            # replicate-1: per-half sums on DVE right after
                            # each square so the final chunk's tail is short
                            nc.vector.reduce_sum(
                                ostage[:, g * K + cb * 8:g * K + cb * 8 + 8],
                                et[(nh, bc)][:, sl].rearrange(
                                    "p (k c) -> p k c", c=d),
                                axis=mybir.AxisListType.X)
            with tc.high_priority(offset=10000):
                nc.sync.dma_start(outp.ap()[:, 64:128], ostage[:, 64:128])

    nc.compile()
    return nc


def _prep(x, Us):
    # fold chol(I - 0.5 U^T U) into U, then scale+quantize to fp8 e4m3
    Us64 = Us.astype(np.float64)
    G = np.einsum('skDa,skDb->skab', Us64, Us64)
    L = np.linalg.cholesky(np.eye(d)[None, None] - 0.5 * G)
    Ut = np.einsum('skDa,skab->skDb', Us64, L)                # (R,K,D,d)
    u8 = (Ut * SCALE).astype(np.float32).astype(ml_dtypes.float8_e4m3)
    x8 = np.ascontiguousarray(x.T).astype(ml_dtypes.float8_e4m3)  # (D, B)

    def u_img(r):  # one replicate -> [128, 2*NJ*2*256] (p, cb, j, i, c)
        ur = np.ascontiguousarray(u8[r].transpose(1, 0, 2)).reshape(D, K * d)
        return np.ascontiguousarray(
            ur.reshape(NJ, 2, 128, 2, 256).transpose(2, 3, 0, 1, 4)
        ).reshape(128, 2 * NJ * 2 * 256)

    def x_img(b4):  # one batch quarter -> [128, BC*NJ*2*128] (p, bc, j, i, n)
        xc = x8[:, NB * b4: NB * (b4 + 1)]                    # (D, 512)
        return np.ascontiguousarray(
            xc.reshape(NJ, 2, 128, BC, 128).transpose(2, 3, 0, 1, 4)
        ).reshape(128, BC * NJ * 2 * 128)

    u_imgs = [u_img(r) for r in range(R)]
    x_imgs = [x_img(b4) for b4 in range(BC)]
    in_maps = []
    for c in range(NCORES):
        s2, b4 = c // 4, c % 4
        in_maps.append({
            "xt": x_imgs[b4],
            "u0": u_imgs[2 * s2],
            "u1": u_imgs[2 * s2 + 1],
        })
    return in_maps


def kernel(x, Us, _trace=False):
    global LAST_RESULTS
    x = np.asarray(x)
    Us = np.asarray(Us)
    if "nc" not in _COMPILED:
        _COMPILED["nc"] = _build()
    nc = _COMPILED["nc"]
    in_maps = _prep(x, Us)
    res = run_bass_kernel_spmd(nc, in_maps, core_ids=list(range(NCORES)),
                               trace=_trace)
    LAST_RESULTS = res
    # base term: exact fp64 sum of squares on host (tiny vs the device work)
    base = 0.5 * float(np.sum(x.astype(np.float64) ** 2)) / B
    obj = np.empty(R, np.float32)
    for r in range(R):
        s2, nh = r // 2, r % 2
        # group col block = (nh*4 + bc)*16; z~ was scaled by SCALE
        terms = []
        for b4 in range(4):
            o = res.results[4 * s2 + b4]["outp"]              # [128, 128]
            for bc in range(BC):
                g = nh * 4 + bc
                terms.append(o[:, g * K:(g + 1) * K].astype(np.float64)
                             .max(axis=1))
        term = np.mean(np.stack(terms)) / (SCALE * SCALE)
        obj[r] = np.float32(base - term)
    return obj


# revision 33
# speedup vs baseline: 1.1702x; 1.0275x over previous
"""Trainium2 Bass kernel for the KSubspaceBaseModel objective.

Reference computes, for B=2048 samples x (B, D=1024) and subspace bases
Us (R=4, K=16, D, d=32):
    z = x @ U; x_ = z @ U^T; loss = 0.5*||x - x_||^2  (per b, r, k)
    obj_r = mean_b min_k loss

Algebraic collapse: with G = U^T U and L = chol(I - 0.5 G) folded host-side
(Ut = U @ L), loss = 0.5||x||^2 - ||Ut^T x||^2, so the device computes
z~ = Ut^T x, squares, and sums each subspace's 32 latent columns; the
per-k sums ship out and the host takes max_k and the fp64 base term.

Device speed notes:
  * fp8 e4m3 operands (Ut scaled by 4096 to stay in the normal range) with
    DoubleRow matmuls: each instruction consumes TWO 128-deep contraction
    chunks ([128, 2, M] stationary x [128, 2, N] moving), 2x PE throughput
    and half the DMA bytes.
  * COLUMN-major u chunking in CONSUMPTION order: each 256 KB u transfer
    carries all 4 contraction pairs for a 256-column block, and the sync
    ring's FIFO [u0c0, u0c1, u1c0, u1c1] means every chunk that lands
    unlocks 16 matmuls immediately (DMA engines round-robin across rings,
    so in-flight transfers finish roughly in issue order).
  * one PSUM bank per (nh, bc) group: matmuls never wait on an epilogue
    reader, keeping the stream gap-free so the PE p-state ramp
    (0.65 -> 1.2 -> 2.4 GHz after a few us continuous) works for us; fp8
    warm-up matmuls bridge the DMA head.
  * epilogue spread over three engines: ScalarE squares (PSUM -> bf16),
    GpSimd sums replicate-0 groups, DVE sums replicate-1 groups per
    half-block so the last chunk's tail is short; k-max and batch-mean
    happen on host from the shipped [128, 8*16] per-k sums.
"""

import numpy as np
import ml_dtypes

import concourse.bass as bass
import concourse.bacc as bacc
import concourse.mybir as mybir
import concourse.tile as tile
from concourse.bass_utils import run_bass_kernel_spmd

B, D, R, K, d = 2048, 1024, 4, 16, 32
NCORES = 8
NB = B // 4          # 512 samples per core
BC = NB // 128       # 4 batch chunks per core
NJ = 4               # contraction pairs (8 kc chunks, 2 per DoubleRow matmul)
SCALE = 4096.0       # Ut pre-scale so fp8 e4m3 values are normal-range
WARM = 15            # PE warm-up matmuls (p-state ramp during DMA head)

FP8 = mybir.dt.float8e4
BF16 = mybir.dt.bfloat16
FP32 = mybir.dt.float32

_COMPILED = {}
LAST_RESULTS = None


def _build():
    nc = bacc.Bacc("TRN2", target_bir_lowering=False, debug=False)
    # host pre-arranges every tensor into its exact SBUF image so each
    # partition's DMA read is one contiguous run
    xt = nc.dram_tensor("xt", [128, BC * NJ * 2 * 128], FP8,
                        kind="ExternalInput")        # [p, bc, j, i, b]
    u0 = nc.dram_tensor("u0", [128, 2 * NJ * 2 * 256], FP8,
                        kind="ExternalInput")        # [p, cb, j, i, c]
    u1 = nc.dram_tensor("u1", [128, 2 * NJ * 2 * 256], FP8,
                        kind="ExternalInput")
    outp = nc.dram_tensor("outp", [128, 128], FP32, kind="ExternalOutput")

    xt_v = xt.ap().rearrange("p (b j i n) -> p b j i n", b=BC, j=NJ, i=2)
    u_v = [u.ap().rearrange("p (cb j i c) -> p cb j i c", cb=2, j=NJ, i=2)
           for u in (u0, u1)]
    DR = mybir.MatmulPerfMode.DoubleRow

    # phase order = chunk consumption order on the sync ring
    PHASES = [(0, 0), (0, 1), (1, 0), (1, 1)]

    with tile.TileContext(nc) as tc:
        with (
            tc.tile_pool(name="xsb", bufs=1) as xpool,
            tc.tile_pool(name="usb", bufs=1) as upool,
            tc.tile_pool(name="esb", bufs=1) as epool,
            tc.tile_pool(name="single", bufs=1) as spool,
            tc.tile_pool(name="zp", bufs=1, space="PSUM") as zpool,
        ):
            # x lands as ONE 512 KB transfer: all batch chunks unblock at
            # once, so the scheduler sees the four bc-groups of every phase
            # as symmetric and orders the epilogue squares early (their
            # semaphore positions then stay small). u chunks go in
            # consumption order; the last is split so it gates only 8
            # matmuls.
            xb = xpool.tile([128, BC, NJ, 2, 128], FP8, tag="x", name="xb")
            u00 = upool.tile([128, NJ, 2, 256], FP8, tag="u00", name="u00")
            u01 = upool.tile([128, NJ, 2, 256], FP8, tag="u01", name="u01")
            u10 = upool.tile([128, NJ, 2, 256], FP8, tag="u10", name="u10")
            u11 = [upool.tile([128, 2, 2, 256], FP8, tag=f"u11{h}",
                              name=f"u11{h}") for h in range(2)]

            def uview(nh, cb, j):
                if (nh, cb) == (0, 0):
                    return u00[:, j]
                if (nh, cb) == (0, 1):
                    return u01[:, j]
                if (nh, cb) == (1, 0):
                    return u10[:, j]
                return u11[j // 2][:, j % 2]

            def xview(bc, j):
                return xb[:, bc, j]

            warm = spool.tile([128, 2, 384], FP8, tag="warm")
            nc.gpsimd.memset(warm[:], 0.0)

            # scalar ring: x as one 512 KB transfer (all batch chunks
            # unblock at once, keeping group readiness symmetric); sync
            # ring: u chunks in consumption order, the last split so it
            # gates only 8 matmuls
            nc.scalar.dma_start(xb[:], xt_v[:])
            nc.sync.dma_start(u00[:], u_v[0][:, 0])
            nc.sync.dma_start(u01[:], u_v[0][:, 1])
            nc.sync.dma_start(u10[:], u_v[1][:, 0])
            nc.sync.dma_start(u11[0][:], u_v[1][:, 1, 0:2])
            nc.sync.dma_start(u11[1][:], u_v[1][:, 1, 2:4])

            # per-k sums land here slice-wise: group g = nh*4+bc covers
            # cols [g*16, (g+1)*16)
            ostage = spool.tile([128, 128], FP32, tag="os", name="ostage")

            # PSUM bank keyed (cb, bc), region keyed nh: phases alternate
            # banks (P1/P3 share a bank, P2/P4 the other), so the only
            # write-after-read pair on a bank is two phases apart and its
            # quantized semaphore wait is satisfied long before it matters
            zps = {(cb, bc): zpool.tile([128, 512], FP32,
                                        tag=f"zp{cb}{bc}",
                                        name=f"zp{cb}{bc}")
                   for cb in range(2) for bc in range(BC)}

            # warm-up: keep the PE busy while the first chunks stream in so
            # the p-state ramp clock starts early (writes bank (0,0); the
            # real group 0 resets it with start=True)
            for _ in range(WARM):
                nc.tensor.matmul(zps[(0, 0)][:, 0:256], warm[:, :, 0:128],
                                 warm[:, :, 128:384], start=True, stop=True,
                                 perf_mode=DR, skip_group_check=True)

            et = {(nh, bc): epool.tile([128, 512], BF16, tag=f"e{nh}{bc}",
                                       name=f"e{nh}{bc}")
                  for nh in range(2) for bc in range(BC)}

            for nh, cb in PHASES:
                sl = slice(cb * 256, (cb + 1) * 256)
                psl = slice(nh * 256, (nh + 1) * 256)
                for bc in range(BC):
                    g = nh * 4 + bc
                    for j in range(NJ):
                        nc.tensor.matmul(
                            zps[(cb, bc)][:, psl], xview(bc, j),
                            uview(nh, cb, j),
                            start=(j == 0), stop=(j == NJ - 1),
                            perf_mode=DR, skip_group_check=True)
                    # epilogue at priority 0: the scheduler places each op
                    # immediately after its producer, so the cross-engine
                    # semaphore targets stay minimal even though the cost
                    # model mispredicts matmul/DMA durations
                    with tc.high_priority(offset=10000):
                        nc.scalar.square(et[(nh, bc)][:, sl],
                                         zps[(cb, bc)][:, psl])
                        # per-half subspace sums on DVE right after each
                        # square: sums pipeline inside every phase and the
                        # final chunk's tail chain is just square + 8-col sum
                        nc.vector.reduce_sum(
                            ostage[:, g * K + cb * 8:g * K + cb * 8 + 8],
                            et[(nh, bc)][:, sl].rearrange(
                                "p (k c) -> p k c", c=d),
                            axis=mybir.AxisListType.X)
                        if (nh, cb, bc) == (0, 1, BC - 1):
                            nc.sync.dma_start(outp.ap()[:, 0:64],
                                              ostage[:, 0:64])
            with tc.high_priority(offset=10000):
                nc.sync.dma_start(outp.ap()[:, 64:128], ostage[:, 64:128])

    nc.compile()
    return nc


def _prep(x, Us):
    # fold chol(I - 0.5 U^T U) into U, then scale+quantize to fp8 e4m3
    Us64 = Us.astype(np.float64)
    G = np.einsum('skDa,skDb->skab', Us64, Us64)
    L = np.linalg.cholesky(np.eye(d)[None, None] - 0.5 * G)
    Ut = np.einsum('skDa,skab->skDb', Us64, L)                # (R,K,D,d)
    u8 = (Ut * SCALE).astype(np.float32).astype(ml_dtypes.float8_e4m3)
    x8 = np.ascontiguousarray(x.T).astype(ml_dtypes.float8_e4m3)  # (D, B)

    def u_img(r):  # one replicate -> [128, 2*NJ*2*256] (p, cb, j, i, c)
        ur = np.ascontiguousarray(u8[r].transpose(1, 0, 2)).reshape(D, K * d)
        return np.ascontiguousarray(
            ur.reshape(NJ, 2, 128, 2, 256).transpose(2, 3, 0, 1, 4)
        ).reshape(128, 2 * NJ * 2 * 256)

    def x_img(b4):  # one batch quarter -> [128, BC*NJ*2*128] (p, bc, j, i, n)
        xc = x8[:, NB * b4: NB * (b4 + 1)]                    # (D, 512)
        return np.ascontiguousarray(
            xc.reshape(NJ, 2, 128, BC, 128).transpose(2, 3, 0, 1, 4)
        ).reshape(128, BC * NJ * 2 * 128)

    u_imgs = [u_img(r) for r in range(R)]
    x_imgs = [x_img(b4) for b4 in range(BC)]
    in_maps = []
    for c in range(NCORES):
        s2, b4 = c // 4, c % 4
        in_maps.append({
            "xt": x_imgs[b4],
            "u0": u_imgs[2 * s2],
            "u1": u_imgs[2 * s2 + 1],
        })
    return in_maps


def kernel(x, Us, _trace=False):
    global LAST_RESULTS
    x = np.asarray(x)
    Us = np.asarray(Us)
    if "nc" not in _COMPILED:
        _COMPILED["nc"] = _build()
    nc = _COMPILED["nc"]
    in_maps = _prep(x, Us)
    res = run_bass_kernel_spmd(nc, in_maps, core_ids=list(range(NCORES)),
                               trace=_trace)
    LAST_RESULTS = res
    # base term: exact fp64 sum of squares on host (tiny vs the device work)
    base = 0.5 * float(np.sum(x.astype(np.float64) ** 2)) / B
    obj = np.empty(R, np.float32)
    for r in range(R):
        s2, nh = r // 2, r % 2
        # group col block = (nh*4 + bc)*16; z~ was scaled by SCALE
        terms = []
        for b4 in range(4):
            o = res.results[4 * s2 + b4]["outp"]              # [128, 128]
            for bc in range(BC):
                g = nh * 4 + bc
                terms.append(o[:, g * K:(g + 1) * K].astype(np.float64)
                             .max(axis=1))
        term = np.mean(np.stack(terms)) / (SCALE * SCALE)
        obj[r] = np.float32(base - term)
    return obj
